# revision 13
# baseline (speedup 1.0000x reference)
"""GQA decoder attention (B=2,T=2048,HID=1024,H=16,HK=4,D=64) on 8 TRN2 cores.

Sharding: core c = 4*b + g handles batch b, kv-head g (q heads 4g..4g+3).
Host pre-transposes hidden/weights and casts to bf16. On chip per core:
  fused QKV proj (bf16 matmuls) -> RMSNorm (one batched sqrt) -> RoPE (DVE,
  f32) -> PE transposes -> causal attention with transposed scores
  (softmax scale folded into exp, rowsum via ones-column in V) ->
  o_proj partial with per-core Wo column slice -> 4 chunked ReduceScatters
  over the 4-core batch group -> [512,1024] f32 shard; host reassembles.
"""
import os
import sys

sys.path.insert(0, "/opt/trn_rl_repo")

import numpy as np
import ml_dtypes

B, T, HID = 2, 2048, 1024
H, HK, D = 16, 4, 64
G = H // HK          # q heads per kv head = 4
EPS = 1e-6
NCORES = 8
NT = T // 128        # 16 t-tiles
HC = HID // 128      # 8 hid chunks
NQT = T // 512       # 4 q-tiles of 512
MASK_VAL = -1e9
QKV = G * D + 2 * D  # 384 fused proj width

_cache = {}


def _build(trace):
    PH = os.environ.get("KERNEL_PHASES", "ARBC")
    import concourse.bass as bass
    import concourse.bacc as bacc
    import concourse.tile as tile
    import concourse.mybir as mybir
    from concourse.alu_op_type import AluOpType
    from concourse import masks

    f32 = mybir.dt.float32
    bf16 = mybir.dt.bfloat16
    Exp = mybir.ActivationFunctionType.Exp
    Sqrt = mybir.ActivationFunctionType.Sqrt
    X = mybir.AxisListType.X

    nc = bacc.Bacc(None, target_bir_lowering=False)

    ht_d = nc.declare_dram_parameter("ht", [HID, T], bf16, isOutput=False)
    wqkvt_d = nc.declare_dram_parameter("wqkvt", [HID, QKV], bf16, isOutput=False)
    wot_d = nc.declare_dram_parameter("wot", [G * D, HID], bf16, isOutput=False)
    csr_d = nc.declare_dram_parameter("csr", [T, 4 * 32], f32, isOutput=False)
    snr_d = nc.declare_dram_parameter("snr", [T, 4 * 32], f32, isOutput=False)
    ident_d = nc.declare_dram_parameter("ident", [128, 128], bf16, isOutput=False)
    mask_d = nc.declare_dram_parameter("mask", [128, 128], f32, isOutput=False)
    ones_d = nc.declare_dram_parameter("ones", [128, NT], bf16, isOutput=False)
    out_d = nc.declare_dram_parameter("out", [512, HID], f32, isOutput=True)

    with tile.TileContext(nc) as tc:
        with (
            tc.tile_pool(name="big", bufs=1) as big,
            tc.tile_pool(name="dram", bufs=1, space="DRAM") as dram,
            tc.tile_pool(name="ps", bufs=1, space="PSUM") as ps,
            tc.tile_pool(name="work", bufs=3) as work,
            tc.tile_pool(name="pt", bufs=8) as ptp,
            tc.tile_pool(name="outp", bufs=2) as outp,
        ):
            # ---- persistent SBUF tensors ----
            ht_sb = big.tile([128, HC, T], bf16)
            wqkvt_sb = big.tile([128, HC, QKV], bf16)
            wot_sb = big.tile([128, 2, HID], bf16)
            csr_sb = big.tile([128, NT, 128], f32)
            snr_sb = big.tile([128, NT, 128], f32)
            qkv_sb = big.tile([128, NT, QKV], f32)     # f32 proj copy
            qrot_sb = big.tile([128, NT, G, D], bf16)  # [t128, j, h, d]
            krot_sb = big.tile([128, NT, D], bf16)
            v_sb = big.tile([128, NT, D + 1], bf16)    # ones col at d=64
            qT_sb = big.tile([64, G, T], bf16)         # [d, h, t]
            kT_sb = big.tile([64, T], bf16)
            ss_sb = big.tile([128, NT, G + 1], f32)
            u_sb = big.tile([128, NT * (G + 1)], f32)
            inv_sb = big.tile([128, NT * (G + 1)], f32)
            att_sb = big.tile([128, NT, G, D], bf16)   # [t128, J, h, d]
            ident = big.tile([128, 128], bf16)
            trimask = big.tile([128, 128], f32)

            rs_in = dram.tile([T, HID], bf16)
            rs_out = [dram.tile([128, HID], bf16, tag=f"rso{m}", name=f"rso{m}")
                      for m in range(4)]

            nc.sync.dma_start(ident[:], ident_d[:])
            nc.sync.dma_start(trimask[:], mask_d[:])
            nc.sync.dma_start(v_sb[:, :, D], ones_d[:])

            # ---- input DMAs ----
            nc.sync.dma_start(ht_sb[:], ht_d[:].rearrange("(c p) t -> p c t", p=128))
            nc.sync.dma_start(wqkvt_sb[:], wqkvt_d[:].rearrange("(c p) d -> p c d", p=128))
            nc.sync.dma_start(wot_sb[:], wot_d[:].rearrange("(c p) d -> p c d", p=128))
            nc.sync.dma_start(csr_sb[:], csr_d[:].rearrange("(j p) d -> p j d", p=128))
            nc.sync.dma_start(snr_sb[:], snr_d[:].rearrange("(j p) d -> p j d", p=128))

            def mixtile(k, shape, dtype):
                return ps.tile(shape, dtype, tag=f"m{k % 2}", name=f"mix{k}")

            mixk = 0

            # ---- phase A: fused qkv projections + sumsq ----
            for j in range(NT if "A" in PH else 0):
                pp = mixtile(mixk, [128, 512], f32)
                mixk += 1
                for i in range(HC):
                    nc.tensor.matmul(pp[:, 0:QKV], ht_sb[:, i, j * 128:(j + 1) * 128],
                                     wqkvt_sb[:, i, :], start=(i == 0), stop=(i == HC - 1))
                nc.vector.tensor_copy(qkv_sb[:, j, :], pp[:, 0:QKV])
                sq = work.tile([128, G * D], f32, tag="sq")
                nc.vector.tensor_mul(sq[:], qkv_sb[:, j, 0:G * D], qkv_sb[:, j, 0:G * D])
                nc.vector.reduce_sum(ss_sb[:, j, 0:G],
                                     sq[:].rearrange("p (h d) -> p h d", d=D), axis=X)
                skv = work.tile([128, D], f32, tag="skv")
                nc.vector.tensor_mul(skv[:], qkv_sb[:, j, G * D:G * D + D],
                                     qkv_sb[:, j, G * D:G * D + D])
                nc.vector.reduce_sum(ss_sb[:, j, G:G + 1],
                                     skv[:].rearrange("p (o d) -> p o d", o=1), axis=X)

            if "A" not in PH:
                nc.gpsimd.memset(ss_sb[:], 1.0)
                nc.gpsimd.memset(qkv_sb[:], 0.5)
            # ---- one batched rsqrt ----
            nc.vector.tensor_scalar(u_sb[:], ss_sb[:].rearrange("p a b -> p (a b)"),
                                    1.0 / D, EPS, op0=AluOpType.mult, op1=AluOpType.add)
            nc.scalar.activation(u_sb[:], u_sb[:], Sqrt)
            nc.vector.reciprocal(inv_sb[:], u_sb[:])
            inv_v = inv_sb[:].rearrange("p (j f) -> p j f", f=G + 1)

            # ---- phase A2: rope + norm-apply + V copy + transposes ----
            for j in range(NT if "R" in PH else 0):
                qv = qkv_sb[:, j, 0:G * D].rearrange("p (h two d) -> p h two d", two=2, d=32)
                c4 = csr_sb[:, j, :].rearrange("p (h d) -> p h d", d=32)
                s4 = snr_sb[:, j, :].rearrange("p (h d) -> p h d", d=32)
                t1 = work.tile([128, G, 32], f32, tag="t1")
                t2 = work.tile([128, G, 32], f32, tag="t2")
                o1 = work.tile([128, G, 32], f32, tag="o1")
                o2 = work.tile([128, G, 32], f32, tag="o2")
                nc.vector.tensor_mul(t1[:], qv[:, :, 0, :], c4[:])
                nc.vector.tensor_mul(t2[:], qv[:, :, 1, :], s4[:])
                nc.vector.tensor_sub(o1[:], t1[:], t2[:])
                nc.vector.tensor_mul(t1[:], qv[:, :, 0, :], s4[:])
                nc.vector.tensor_mul(t2[:], qv[:, :, 1, :], c4[:])
                nc.vector.tensor_add(o2[:], t1[:], t2[:])
                qr = qrot_sb[:, j, :, :].rearrange("p h (two d) -> p h two d", two=2)
                for h in range(G):
                    nc.vector.tensor_scalar_mul(qr[:, h, 0, :], o1[:, h, :], inv_v[:, j, h:h + 1])
                    nc.vector.tensor_scalar_mul(qr[:, h, 1, :], o2[:, h, :], inv_v[:, j, h:h + 1])
                # k rope (1 head)
                kv2 = qkv_sb[:, j, G * D:G * D + D].rearrange("p (two d) -> p two d", two=2)
                kt1 = work.tile([128, 32], f32, tag="kt1")
                kt2 = work.tile([128, 32], f32, tag="kt2")
                ko1 = work.tile([128, 32], f32, tag="ko1")
                ko2 = work.tile([128, 32], f32, tag="ko2")
                nc.vector.tensor_mul(kt1[:], kv2[:, 0, :], c4[:, 0, :])
                nc.vector.tensor_mul(kt2[:], kv2[:, 1, :], s4[:, 0, :])
                nc.vector.tensor_sub(ko1[:], kt1[:], kt2[:])
                nc.vector.tensor_mul(kt1[:], kv2[:, 0, :], s4[:, 0, :])
                nc.vector.tensor_mul(kt2[:], kv2[:, 1, :], c4[:, 0, :])
                nc.vector.tensor_add(ko2[:], kt1[:], kt2[:])
                kr = krot_sb[:, j, :].rearrange("p (two d) -> p two d", two=2)
                nc.vector.tensor_scalar_mul(kr[:, 0, :], ko1[:], inv_v[:, j, G:G + 1])
                nc.vector.tensor_scalar_mul(kr[:, 1, :], ko2[:], inv_v[:, j, G:G + 1])
                nc.vector.tensor_copy(v_sb[:, j, 0:D], qkv_sb[:, j, G * D + D:QKV])

                for h in range(G):
                    pt = mixtile(mixk, [128, 128], bf16)
                    mixk += 1
                    nc.tensor.transpose(pt[0:64, :], qrot_sb[:, j, h, :], ident[:])
                    nc.vector.tensor_copy(qT_sb[:, h, j * 128:(j + 1) * 128], pt[0:64, :])
                pk = mixtile(mixk, [128, 128], bf16)
                mixk += 1
                nc.tensor.transpose(pk[0:64, :], krot_sb[:, j, :], ident[:])
                nc.vector.tensor_copy(kT_sb[:, j * 128:(j + 1) * 128], pk[0:64, :])

            # ---- phase B: attention ----
            scale = 1.0 / np.sqrt(D)
            sk = 0
            if "R" not in PH:
                nc.gpsimd.memset(qT_sb[:], 0.01)
                nc.gpsimd.memset(kT_sb[:], 0.01)
                nc.gpsimd.memset(v_sb[:], 0.01)
            for j in range(NQT if "B" in PH else 0):
                nchunk = 4 * j + 4
                for h in range(G):
                    pts = []
                    for g0 in range(0, nchunk, 2):
                        sps = ps.tile([128, 2, 512], f32, tag=f"s{sk % 2}",
                                      name=f"sps{sk}")
                        sk += 1
                        pt = ptp.tile([128, 2, 512], bf16, tag="pt")
                        for ii in range(2):
                            i = g0 + ii
                            m = i - 4 * j
                            x0 = 128 * m if m > 0 else 0
                            nc.tensor.matmul(
                                sps[:, ii, x0:512],
                                kT_sb[:, i * 128:(i + 1) * 128],
                                qT_sb[:, h, j * 512 + x0:(j + 1) * 512],
                                start=True, stop=True)
                            if m >= 0:
                                nc.vector.tensor_add(sps[:, ii, 128 * m:128 * m + 128],
                                                     sps[:, ii, 128 * m:128 * m + 128],
                                                     trimask[:])
                        nc.scalar.activation(pt[:].rearrange("p a b -> p (a b)"),
                                             sps[:].rearrange("p a b -> p (a b)"),
                                             Exp, scale=scale)
                        pts.append(pt)
                    aps = ps.tile([128, 4, D + 1], f32, tag="att", name=f"att{j}_{h}")
                    for mp in range(4):
                        last = 4 * j + mp
                        for i in range(last + 1):
                            nc.tensor.matmul(
                                aps[:, mp, :],
                                pts[i // 2][:, i % 2, mp * 128:(mp + 1) * 128],
                                v_sb[:, i, :],
                                start=(i == 0), stop=(i == last))
                        J = 4 * j + mp
                        invd = work.tile([128, 1], f32, tag="invd")
                        nc.vector.reciprocal(invd[:], aps[:, mp, D:D + 1])
                        nc.vector.tensor_scalar_mul(att_sb[:, J, h, :], aps[:, mp, 0:D],
                                                    invd[:])

            # ---- phase C: o_proj partials + 4 chunked ReduceScatters ----
            rg = [[0, 1, 2, 3], [4, 5, 6, 7]]
            if "B" not in PH:
                nc.gpsimd.memset(att_sb[:], 0.01)
            for jj in range(NT if "C" in PH else 0):
                attT = outp.tile([128, 2, 128], bf16, tag="attT")
                attf = att_sb[:, jj, :, :].rearrange("p h d -> p (h d)")
                for i in range(2):
                    pt = mixtile(mixk, [128, 128], bf16)
                    mixk += 1
                    nc.tensor.transpose(pt[:], attf[:, i * 128:(i + 1) * 128], ident[:])
                    nc.vector.tensor_copy(attT[:, i, :], pt[:])
                o_sb = outp.tile([128, HID], bf16, tag="osb")
                for n in range(2):
                    ops = mixtile(mixk, [128, 512], f32)
                    mixk += 1
                    for i in range(2):
                        nc.tensor.matmul(ops[:], attT[:, i, :],
                                         wot_sb[:, i, n * 512:(n + 1) * 512],
                                         start=(i == 0), stop=(i == 1))
                    nc.vector.tensor_copy(o_sb[:, n * 512:(n + 1) * 512], ops[:])
                nc.sync.dma_start(rs_in[jj * 128:(jj + 1) * 128, :], o_sb[:])
                if jj % 4 == 3:
                    m = jj // 4
                    nc.gpsimd.collective_compute(
                        "ReduceScatter", AluOpType.add,
                        replica_groups=rg,
                        ins=[rs_in[m * 512:(m + 1) * 512, :]],
                        outs=[rs_out[m].opt()],
                    )
                    ob = outp.tile([128, HID], bf16, tag="ob")
                    of = outp.tile([128, HID], f32, tag="of")
                    nc.sync.dma_start(ob[:], rs_out[m].opt())
                    nc.vector.tensor_copy(of[:], ob[:])
                    nc.sync.dma_start(out_d[m * 128:(m + 1) * 128, :], of[:])

    nc.compile()
    return nc


def _get_nc(trace):
    key = ("nc", trace)
    if key not in _cache:
        _cache[key] = _build(trace)
    return _cache[key]


def _install_ntff_hook():
    """Create the missing antenv.axon_hooks module driving NTFF profiling
    via ctypes into libaxon_pjrt.so (same recipe as trn_boot.py)."""
    import types
    import ctypes
    import contextlib

    if "antenv.axon_hooks" in sys.modules:
        return
    so_path = "/opt/axon/libaxon_pjrt.so"
    if not os.path.exists(so_path):
        return
    lib = ctypes.CDLL(so_path)
    if not hasattr(lib, "axon_start_nrt_profile"):
        return
    lib.axon_start_nrt_profile.argtypes = [ctypes.POINTER(ctypes.c_int64),
                                           ctypes.c_size_t]
    lib.axon_start_nrt_profile.restype = ctypes.c_int64
    lib.axon_stop_nrt_profile.argtypes = [ctypes.c_char_p]
    lib.axon_stop_nrt_profile.restype = ctypes.c_int64

    @contextlib.contextmanager
    def _hook(output_dir, device_ids=None):
        import jax
        jax.devices()
        if device_ids:
            ids = (ctypes.c_int64 * len(device_ids))(*device_ids)
            rc = lib.axon_start_nrt_profile(ids, len(device_ids))
        else:
            rc = lib.axon_start_nrt_profile(None, 0)
        if rc != 0:
            raise RuntimeError(f"axon_start_nrt_profile rc={rc}")
        try:
            yield
        finally:
            n = lib.axon_stop_nrt_profile(str(output_dir).encode())
            print(f"profile: {n} file(s) written to {output_dir}",
                  file=sys.stderr)

    mod = types.ModuleType("antenv.axon_hooks")
    mod.get_axon_ntff_profile_hook = lambda: _hook
    mod.set_axon_ntff_profile_hook = lambda h: None
    sys.modules["antenv.axon_hooks"] = mod
    import antenv
    antenv.axon_hooks = mod


def kernel(hidden_states, cos, sin, Wq, Wk, Wv, Wo, q_norm_w, k_norm_w):
    from concourse.bass_utils import run_bass_kernel_spmd

    trace = bool(int(os.environ.get("KERNEL_TRACE", "0")))
    if trace:
        try:
            _install_ntff_hook()
        except Exception as e:
            print(f"ntff hook install failed: {e}", file=sys.stderr)
    nc = _get_nc(trace)

    bf = ml_dtypes.bfloat16
    hidden_states = np.asarray(hidden_states, np.float32)
    cos = np.asarray(cos, np.float32).reshape(T, 32)
    sin = np.asarray(sin, np.float32).reshape(T, 32)
    Wq = np.asarray(Wq, np.float32)
    Wk = np.asarray(Wk, np.float32)
    Wv = np.asarray(Wv, np.float32)
    Wo = np.asarray(Wo, np.float32)

    csr = np.tile(cos, (1, 4)).astype(np.float32)
    snr = np.tile(sin, (1, 4)).astype(np.float32)
    ident_np = np.eye(128, dtype=bf)
    mask_np = np.where(np.arange(128)[:, None] > np.arange(128)[None, :],
                       np.float32(MASK_VAL), np.float32(0.0))
    ones_np = np.ones((128, NT), dtype=bf)

    in_maps = []
    for c in range(NCORES):
        b, g = c // 4, c % 4
        ht = np.ascontiguousarray(hidden_states[b].T).astype(bf)
        wqkvt = np.ascontiguousarray(
            np.concatenate([Wq[g * G * D:(g + 1) * G * D, :].T,
                            Wk[g * D:(g + 1) * D, :].T,
                            Wv[g * D:(g + 1) * D, :].T], axis=1)).astype(bf)
        wot = np.ascontiguousarray(Wo[:, g * G * D:(g + 1) * G * D].T).astype(bf)
        in_maps.append({"ht": ht, "wqkvt": wqkvt, "wot": wot,
                        "csr": csr, "snr": snr, "ident": ident_np,
                        "mask": mask_np, "ones": ones_np})

    res = run_bass_kernel_spmd(nc, in_maps, core_ids=list(range(NCORES)),
                               trace=trace)
    kernel.last_exec_time_ns = res.exec_time_ns

    out = np.zeros((B, T, HID), np.float32)
    for c in range(NCORES):
        b, g = c // 4, c % 4
        shard = np.asarray(res.results[c]["out"], np.float32)  # [512, 1024]
        for m in range(4):
            out[b, m * 512 + g * 128:m * 512 + (g + 1) * 128, :] = \
                shard[m * 128:(m + 1) * 128]
    return out


kernel.last_exec_time_ns = None
